# revision 16
# baseline (speedup 1.0000x reference)
"""Bass/Trainium2 kernel for nn_BakaAttention: 8-way data-parallel over batch.

Per core (one batch element):
  q = rope(x@wq, off=1024); k = rope(concat(past_k, x@wk), off=0); v = concat(past_v, x@wv)
  out = softmax(mask(q k^T / 16)) v @ wo

Layouts on chip: qT/kT are feature-major [f, t]; scores computed transposed
[s, t] so PV consumes probs directly as the stationary operand. All matmuls
run in float32r (full PE rate at N>=256). Softmax denominators accumulate on
the vector engine (tile adds), reduced across partitions by a single
ones-matmul per tile-block; transposes use the PE transpose path (1.5
cycles/row instead of 4); q stays resident in SBUF between projection and
attention.
"""

import numpy as np

B, T, P, H, DH, DIN, DOUT = 8, 1024, 1024, 4, 256, 1024, 1152
S = P + T  # 2048 keys
THETA = 10000.0
NCORES = 8


def _host_constants():
    m = np.arange(0, DH, 2, dtype=np.float64) / DH          # 128 freqs
    inv = 1.0 / (THETA ** m)                                # [128]
    pos = np.arange(S, dtype=np.float64)                    # [2048]
    ang = np.outer(inv, pos)                                # [128, 2048]
    cos_full = np.cos(ang)
    sin_full = np.sin(ang)
    r = np.arange(128) // 2
    consts = {
        "cos_lo": cos_full[r, :].astype(np.float32),
        "cos_hi": cos_full[64 + r, :].astype(np.float32),
        "sin_lo": sin_full[r, :].astype(np.float32),
        "sin_hi": sin_full[64 + r, :].astype(np.float32),
    }
    prot = np.zeros((128, 128), np.float32)
    for mm in range(64):
        prot[2 * mm, 2 * mm + 1] = 1.0
        prot[2 * mm + 1, 2 * mm] = -1.0
    consts["prot"] = prot
    consts["ident"] = np.eye(128, dtype=np.float32)
    # masks[ci][sl, tl] = 1.0 if sl <= tl - 128*ci else 0 (keep), ci in 0..3
    sl = np.arange(128)[:, None]
    tl = np.arange(512)[None, :]
    masks = np.stack(
        [(sl <= tl - 128 * ci).astype(np.float32) for ci in range(4)], axis=1
    )  # [128, 4, 512]
    consts["masks"] = np.ascontiguousarray(masks)
    consts["ones"] = np.ones((128, 4), np.float32)
    consts["onesr"] = np.ones((1, 128), np.float32)
    return consts


def build_kernel(debug=False):
    import concourse.bass as bass
    import concourse.mybir as mybir
    from concourse import bacc
    from concourse.tile import TileContext

    f32 = mybir.dt.float32
    f32r = mybir.dt.float32r
    AF = mybir.ActivationFunctionType
    OP = mybir.AluOpType

    nc = bacc.Bacc(None, target_bir_lowering=False)

    x_d = nc.dram_tensor("x", [T, DIN], f32r, kind="ExternalInput")
    pk_d = nc.dram_tensor("past_k", [P, H, DH], f32r, kind="ExternalInput")
    pv_d = nc.dram_tensor("past_v", [P, H, DH], f32r, kind="ExternalInput")
    wq_d = nc.dram_tensor("wq", [DIN, DIN], f32r, kind="ExternalInput")
    wk_d = nc.dram_tensor("wk", [DIN, DIN], f32r, kind="ExternalInput")
    wv_d = nc.dram_tensor("wv", [DIN, DIN], f32r, kind="ExternalInput")
    wo_d = nc.dram_tensor("wo", [DIN, DOUT], f32r, kind="ExternalInput")
    cos_lo_d = nc.dram_tensor("cos_lo", [128, S], f32, kind="ExternalInput")
    cos_hi_d = nc.dram_tensor("cos_hi", [128, S], f32, kind="ExternalInput")
    sin_lo_d = nc.dram_tensor("sin_lo", [128, S], f32, kind="ExternalInput")
    sin_hi_d = nc.dram_tensor("sin_hi", [128, S], f32, kind="ExternalInput")
    prot_d = nc.dram_tensor("prot", [128, 128], f32r, kind="ExternalInput")
    ident_d = nc.dram_tensor("ident", [128, 128], f32r, kind="ExternalInput")
    masks_d = nc.dram_tensor("masks", [128, 4, 512], f32, kind="ExternalInput")
    ones_d = nc.dram_tensor("ones", [128, 4], f32r, kind="ExternalInput")
    onesr_d = nc.dram_tensor("onesr", [1, 128], f32r, kind="ExternalInput")
    out_d = nc.dram_tensor("out", [T, DOUT], f32, kind="ExternalOutput")
    vkind = dict(kind="ExternalOutput") if debug else {}
    v_r = nc.dram_tensor("v_r", [T, DIN], f32r, **vkind)
    if debug:
        qT_dump = nc.dram_tensor("qT_dump", [8, 128, T], f32r, kind="ExternalOutput")
        kT_dump = nc.dram_tensor("kT_dump", [8, 128, S], f32r, kind="ExternalOutput")

    from contextlib import ExitStack
    stack = ExitStack()
    with TileContext(nc) as tc, stack:
        cstp = stack.enter_context(tc.tile_pool(name="consts", bufs=1))
        prot = cstp.tile([128, 128], f32r, name="prot", tag="prot")
        ident = cstp.tile([128, 128], f32r, name="ident", tag="ident")
        ones_sb = cstp.tile([128, 4], f32r, name="ones_sb", tag="ones_sb")
        nc.sync.dma_start(out=ones_sb[:], in_=ones_d[:])
        onesr_sb = cstp.tile([1, 128], f32r, name="onesr_sb", tag="onesr_sb")
        nc.sync.dma_start(out=onesr_sb[:], in_=onesr_d[:])
        nc.sync.dma_start(out=prot[:], in_=prot_d[:])
        nc.sync.dma_start(out=ident[:], in_=ident_d[:])

        resid = stack.enter_context(tc.tile_pool(name="resid", bufs=1))
        kT = [resid.tile([128, S], f32r, name=f"kT{i}", tag=f"kT{i}") for i in range(8)]
        qTp = stack.enter_context(tc.tile_pool(name="qTp", bufs=1))
        qT = [qTp.tile([128, T], f32r, name=f"qT{i}", tag=f"qT{i}") for i in range(8)]
        mskp = stack.enter_context(tc.tile_pool(name="p3msk", bufs=1))
        masks = mskp.tile([128, 4, 512], f32, name="masks", tag="masks")
        nc.sync.dma_start(out=masks[:], in_=masks_d[:])

        # ---------------- Phase A-E: xT, projections, rope ----------------
        with tc.tile_pool(name="tables", bufs=1) as tabp, \
             tc.tile_pool(name="p2xT", bufs=1) as xtp, \
             tc.tile_pool(name="p2", bufs=2) as p2p, \
             tc.tile_pool(name="p2st", bufs=3) as stp:
            cos_t = [tabp.tile([128, T], f32, name="clo", tag="clo"),
                     tabp.tile([128, T], f32, name="chi", tag="chi")]
            sin_t = [tabp.tile([128, T], f32, name="slo", tag="slo"),
                     tabp.tile([128, T], f32, name="shi", tag="shi")]

            def load_tables(p0):
                nc.sync.dma_start(out=cos_t[0][:], in_=cos_lo_d[:, p0:p0 + T])
                nc.sync.dma_start(out=cos_t[1][:], in_=cos_hi_d[:, p0:p0 + T])
                nc.sync.dma_start(out=sin_t[0][:], in_=sin_lo_d[:, p0:p0 + T])
                nc.sync.dma_start(out=sin_t[1][:], in_=sin_hi_d[:, p0:p0 + T])

            xT = [xtp.tile([128, T], f32r, name=f"xT{i}", tag=f"xT{i}") for i in range(8)]

            def rope_combine(dst_ap, raw_sb, rot_ps, ft, off, n):
                # dst = raw * cos + rot * sin ; table rows by f-tile parity
                ctab = cos_t[ft % 2][:, off:off + n]
                stab = sin_t[ft % 2][:, off:off + n]
                t1 = p2p.tile([128, 512], f32, name="ropet1", tag="ropet1")
                nc.gpsimd.tensor_tensor(t1[:, :n], raw_sb, ctab, op=OP.mult)
                t2 = p2p.tile([128, 512], f32, name="ropet2", tag="ropet2")
                nc.vector.tensor_tensor(t2[:, :n], rot_ps, stab, op=OP.mult)
                nc.vector.tensor_tensor(dst_ap, t1[:, :n], t2[:, :n], op=OP.add)

            with tc.tile_pool(name="p2rot", bufs=2, space="PSUM") as rotps:
                # -- A: transpose x into xT (PE transpose, 1.5 cyc/row) --
                with tc.tile_pool(name="xldp", bufs=3) as xlp, \
                     tc.tile_pool(name="p2tpa", bufs=4, space="PSUM") as tppa:
                    for tt in range(8):
                        xt = xlp.tile([128, DIN], f32r, name="xload", tag="xload")
                        nc.sync.dma_start(out=xt[:], in_=x_d[128 * tt:128 * (tt + 1), :])
                        for kt in range(8):
                            tp = tppa.tile([128, 128], f32r, name="tps", tag="tps")
                            nc.tensor.transpose(tp[:], xt[:, 128 * kt:128 * (kt + 1)], ident[:])
                            nc.scalar.copy(xT[kt][:, 128 * tt:128 * (tt + 1)], tp[:])
                        if tt == 0:
                            load_tables(P)  # positions 1024..2047 for q, new-k

                # -- B,E: k then q projections (transposed layout) + rope.
                # kt-accumulation runs per f-tile (half-group of 2 PSUM
                # banks); the copy+rotate+combine epilogue of each f-tile is
                # emitted one half-group later so the PE never waits on the
                # PSUM-draining copies.
                pending_rot = None

                def make_rot(dst, ft, psl2):
                    def emit():
                        raw = p2p.tile([128, 1024], f32r, name="rawsb", tag="rawsb")
                        for th in range(2):
                            nc.scalar.copy(raw[:, 512 * th:512 * (th + 1)],
                                           psl2[th][:])
                        for th in range(2):
                            rp = rotps.tile([128, 512], f32, name="rotps", tag="rotps")
                            nc.tensor.matmul(rp[:], prot[:].bitcast(f32r),
                                             raw[:, 512 * th:512 * (th + 1)].bitcast(f32r),
                                             start=True, stop=True)
                            off = P if dst is kT else 0
                            dst_ap = dst[ft][:, off + 512 * th:off + 512 * (th + 1)]
                            rope_combine(dst_ap, raw[:, 512 * th:512 * (th + 1)],
                                         rp[:], ft, 512 * th, 512)
                    return emit

                with tc.tile_pool(name="p2ps", bufs=4, space="PSUM") as ps2:
                    for w_d, dst in ((wk_d, kT), (wq_d, qT)):
                        for ftg in range(4):
                            for f2 in range(2):
                                ft = 2 * ftg + f2
                                psl2 = [ps2.tile([128, 512], f32, name=f"pj{2 * f2 + th}",
                                                 tag=f"pj{2 * f2 + th}", bufs=1)
                                        for th in range(2)]
                                for kt in range(8):
                                    wt = stp.tile([128, 128], f32r, name="wload",
                                                  tag="wload", bufs=8)
                                    nc.sync.dma_start(
                                        out=wt[:],
                                        in_=w_d[128 * kt:128 * (kt + 1),
                                                256 * ftg + 128 * f2:256 * ftg + 128 * (f2 + 1)])
                                    for th in range(2):
                                        nc.tensor.matmul(
                                            psl2[th][:],
                                            wt[:].bitcast(f32r),
                                            xT[kt][:, 512 * th:512 * (th + 1)].bitcast(f32r),
                                            start=(kt == 0), stop=(kt == 7))
                                    if kt == 2 and pending_rot is not None:
                                        pending_rot()
                                        pending_rot = None
                                pending_rot = make_rot(dst, ft, psl2)
                    pending_rot()
                    pending_rot = None

                # -- D: past_k transpose + rope into kT[:, 0:1024] --
                with tc.tile_pool(name="p2kp", bufs=2) as kpp, \
                     tc.tile_pool(name="p2tpd", bufs=4, space="PSUM") as tpd:
                    for h in range(4):
                        kp = [kpp.tile([128, P], f32r, name=f"kp{i}", tag=f"kp{i}")
                              for i in range(2)]
                        for st in range(8):
                            pkt = stp.tile([128, DH], f32r, name="pkload",
                                           tag="pkload", bufs=6)
                            nc.sync.dma_start(out=pkt[:],
                                              in_=pk_d[128 * st:128 * (st + 1), h, :])
                            for f2 in range(2):
                                tp = tpd.tile([128, 128], f32r, name="tps", tag="tps")
                                nc.tensor.transpose(tp[:], pkt[:, 128 * f2:128 * (f2 + 1)],
                                                    ident[:])
                                nc.scalar.copy(kp[f2][:, 128 * st:128 * (st + 1)], tp[:])
                        if h == 0:
                            load_tables(0)  # positions 0..1023
                        for f2 in range(2):
                            ft = 2 * h + f2
                            for sh in range(2):
                                rp = rotps.tile([128, 512], f32, name="rotps", tag="rotps")
                                nc.tensor.matmul(rp[:], prot[:].bitcast(f32r),
                                                 kp[f2][:, 512 * sh:512 * (sh + 1)].bitcast(f32r),
                                                 start=True, stop=True)
                                rope_combine(kT[ft][:, 512 * sh:512 * (sh + 1)],
                                             kp[f2][:, 512 * sh:512 * (sh + 1)],
                                             rp[:], ft, 512 * sh, 512)

            # -- C: v projection, natural layout [s, f] -> DRAM --
            # last; f-half outer so wv is loaded once (resident half in SBUF)
            with tc.tile_pool(name="pvps", bufs=2, space="PSUM") as pvp, \
                 tc.tile_pool(name="wvres", bufs=1) as wvr:
                for fh in range(2):
                    wvh = [wvr.tile([128, 512], f32r, name=f"wvh{kt}", tag=f"wvh{kt}")
                           for kt in range(8)]
                    for kt in range(8):
                        nc.sync.dma_start(
                            out=wvh[kt][:],
                            in_=wv_d[128 * kt:128 * (kt + 1), 512 * fh:512 * (fh + 1)])
                    for stg in range(4):
                        psl = [pvp.tile([128, 512], f32, name=f"pv{s2}", tag=f"pv{s2}")
                               for s2 in range(2)]
                        for kt in range(8):
                            for s2 in range(2):
                                st = 2 * stg + s2
                                nc.tensor.matmul(
                                    psl[s2][:],
                                    xT[kt][:, 128 * st:128 * (st + 1)].bitcast(f32r),
                                    wvh[kt][:],
                                    start=(kt == 0), stop=(kt == 7))
                        for s2 in range(2):
                            st = 2 * stg + s2
                            vsb = p2p.tile([128, 512], f32r, name="vsb", tag="vsb")
                            nc.scalar.copy(vsb[:], psl[s2][:])
                            nc.sync.dma_start(
                                out=v_r[128 * st:128 * (st + 1),
                                        512 * fh:512 * (fh + 1)],
                                in_=vsb[:])

        if debug:
            for i in range(8):
                nc.sync.dma_start(out=kT_dump[i], in_=kT[i][:])
                nc.sync.dma_start(out=qT_dump[i], in_=qT[i][:])

        # ---------------- Phase F: attention ----------------
        ysbp = stack.enter_context(tc.tile_pool(name="ysb", bufs=1))
        yT = [ysbp.tile([128, T], f32r, name=f"yT{i}", tag=f"yT{i}")
              for i in range(8)]
        with tc.tile_pool(name="vaug", bufs=2) as vap, \
             tc.tile_pool(name="probs", bufs=4) as prp, \
             tc.tile_pool(name="p3sm", bufs=2) as smp, \
             tc.tile_pool(name="p3acc", bufs=2) as accp, \
             tc.tile_pool(name="p3sc", bufs=3, space="PSUM") as scps, \
             tc.tile_pool(name="p3smps", bufs=1, space="PSUM") as smps_p, \
             tc.tile_pool(name="p3y", bufs=2, space="PSUM") as yps:

            def load_va(h):
                va = [vap.tile([128, DH], f32r, name=f"va{j}", tag=f"va{j}")
                      for j in range(16)]
                for j in range(16):
                    if j < 8:
                        src = pv_d[128 * j:128 * (j + 1), h, :]
                    else:
                        src = v_r[128 * (j - 8):128 * (j - 7),
                                  DH * h:DH * (h + 1)]
                    nc.sync.dma_start(out=va[j][:, 0:DH], in_=src)
                return va

            va = load_va(0)
            pending_epi = None
            for h in range(4):
                va_next = None
                for TH in range(2):
                    jmax = 12 + 4 * TH
                    ytp_ps = [yps.tile([128, 512], f32, name=f"ytp{i}",
                                       tag=f"ytp{i}") for i in range(2)]
                    acc = accp.tile([128, 512], f32r, name="acc", tag="acc")
                    pj_prev = None
                    for j in range(jmax):
                        sc = scps.tile([128, 512], f32, name="sc", tag="sc")
                        for fk in range(2):
                            nc.tensor.matmul(
                                sc[:],
                                kT[2 * h + fk][:, 128 * j:128 * (j + 1)].bitcast(f32r),
                                qT[2 * h + fk][:, 512 * TH:512 * (TH + 1)].bitcast(f32r),
                                start=(fk == 0), stop=(fk == 1))
                        pj = prp.tile([128, 512], f32r, name="pj", tag="pj")
                        nc.scalar.activation(pj[:], sc[:], AF.Exp, scale=float(DH ** -0.5))
                        ci = j - (8 + 4 * TH)
                        if ci >= 0:
                            nc.gpsimd.tensor_tensor(pj[:], pj[:], masks[:, ci, :],
                                                    op=OP.mult)
                        for fb in range(2):
                            nc.tensor.matmul(
                                ytp_ps[fb][:],
                                va[j][:, 128 * fb:128 * (fb + 1)],
                                pj[:],
                                start=(j == 0), stop=(j == jmax - 1))
                        # denominator accumulation on DVE
                        if j == 1:
                            nc.vector.tensor_tensor(acc[:], pj_prev[:], pj[:],
                                                    op=OP.add)
                        elif j > 1:
                            nc.vector.tensor_tensor(acc[:], acc[:], pj[:],
                                                    op=OP.add)
                        pj_prev = pj
                        # software-pipelined: previous block's epilogue +
                        # next h's v prefetch land early in this j-loop
                        if j == 2 and pending_epi is not None:
                            pending_epi()
                            pending_epi = None
                        if j == 4 and TH == 1 and h < 3 and va_next is None:
                            va_next = load_va(h + 1)

                    def make_epi(h=h, TH=TH, acc=acc, ytp_ps=ytp_ps):
                        def epi():
                            sm = smps_p.tile([1, 512], f32, name="smps", tag="smps")
                            nc.tensor.matmul(sm[:], ones_sb[:, 0:1], acc[:],
                                             start=True, stop=True)
                            rc = smp.tile([1, 512], f32, name="rc", tag="rc")
                            nc.vector.reciprocal_approx_fast(rc[:], sm[:])
                            rcr = smp.tile([1, 512], f32r, name="rcr", tag="rcr")
                            nc.scalar.copy(rcr[:], rc[:])
                            bc = scps.tile([128, 512], f32, name="bc", tag="sc")
                            nc.tensor.matmul(bc[:], onesr_sb[:], rcr[:],
                                             start=True, stop=True)
                            bc_sb = smp.tile([128, 512], f32, name="bcsb",
                                             tag="bcsb")
                            nc.scalar.copy(bc_sb[:], bc[:])
                            for fb in range(2):
                                nc.vector.tensor_tensor(
                                    yT[2 * h + fb][:, 512 * TH:512 * (TH + 1)],
                                    ytp_ps[fb][:],
                                    bc_sb[:],
                                    op=OP.mult)
                        return epi

                    pending_epi = make_epi()
                if va_next is not None:
                    va = va_next
            # final (h=3, TH=1) epilogue: o-proj tiles tt 0..3 don't depend
            # on it, and its PE work is tiny; emit it here before closing.
            pending_epi()

        # ---------------- Phase G: o-projection ----------------
        with tc.tile_pool(name="p4wo", bufs=1) as wop, \
             tc.tile_pool(name="p4o", bufs=2) as osp, \
             tc.tile_pool(name="p4ps", bufs=3, space="PSUM") as ps4:
            wo_sb = [wop.tile([128, DOUT], f32r, name=f"wo{i}", tag=f"wo{i}")
                     for i in range(8)]
            for kt in range(8):
                nc.sync.dma_start(out=wo_sb[kt][:],
                                  in_=wo_d[128 * kt:128 * (kt + 1), :])
            for tt in range(8):
                ot = osp.tile([128, DOUT], f32, name="osb", tag="osb")
                for ds in range(3):
                    op_ps = ps4.tile([128, 384], f32, name="ops", tag="ops", bufs=3)
                    for fk in range(8):
                        nc.tensor.matmul(
                            op_ps[:],
                            yT[fk][:, 128 * tt:128 * (tt + 1)],
                            wo_sb[fk][:, 384 * ds:384 * (ds + 1)],
                            start=(fk == 0), stop=(fk == 7))
                    nc.scalar.copy(ot[:, 384 * ds:384 * (ds + 1)], op_ps[:])
                nc.sync.dma_start(out=out_d[128 * tt:128 * (tt + 1), :], in_=ot[:])

    nc.finalize()
    return nc


_NC_CACHE = {}


def run(x, past_k, past_v, wq, wk, wv, wo, debug=False, trace=False):
    from concourse.bass_utils import run_bass_kernel_spmd

    key = (debug,)
    if key not in _NC_CACHE:
        _NC_CACHE[key] = build_kernel(debug=debug)
    nc = _NC_CACHE[key]
    consts = _host_constants()
    in_maps = []
    for b in range(NCORES):
        m = {
            "x": np.ascontiguousarray(x[b]),
            "past_k": np.ascontiguousarray(past_k[b]),
            "past_v": np.ascontiguousarray(past_v[b]),
            "wq": wq, "wk": wk, "wv": wv, "wo": wo,
            "cos_lo": consts["cos_lo"], "cos_hi": consts["cos_hi"],
            "sin_lo": consts["sin_lo"], "sin_hi": consts["sin_hi"],
            "prot": consts["prot"], "ident": consts["ident"],
            "masks": consts["masks"], "ones": consts["ones"], "onesr": consts["onesr"],
        }
        in_maps.append(m)
    res = run_bass_kernel_spmd(nc, in_maps, list(range(NCORES)), trace=trace)
    out = np.stack([res.results[b]["out"] for b in range(NCORES)], axis=0)
    return out, res


def kernel(x, past_k, past_v, wq, wk, wv, wo):
    out, _ = run(x, past_k, past_v, wq, wk, wv, wo)
    return out
